# revision 1
# baseline (speedup 1.0000x reference)
"""Segment-mean + projection kernel for Trainium2 (8 NeuronCores, SPMD).

logits[b] = (mean of x rows in bag b) @ rel_weight.T + bias

Strategy: data-parallel over bags. Each core gets a bag-aligned slice of
rows, padded to G groups of 768 rows (6 tiles of 128). Per 128-row tile the
DVE builds a one-hot matrix A[p, f] = (seg_local[p] == f) and the PE
accumulates A.T @ x into PSUM over the group's 6 tiles (fp32r matmuls).
Bags split across a group boundary are repaired with a rank-1 fixup matmul
(one-hot row DMA'd from host). Means = PSUM * (1/count) per column, then
PE-transposed into [D, bags] layout and projected against W.T chunks, bias
added, emitted as logitsT [53, bags-slots]; the host compacts the valid
columns. All data-dependent structure travels as DMA'd tensors, so one
program serves all 8 cores.
"""
import sys
import re

sys.path.insert(0, "/opt/trn_rl_repo")

import numpy as np

N_CORES = 8
SERIAL_BUFS = 0  # set 1 to serialize pipeline for debug
ROWS_PER_TILE = 128
TILES_PER_GROUP = 6
ROWS_PER_GROUP = ROWS_PER_TILE * TILES_PER_GROUP  # 768
D = 690
D_SPLIT = 344  # fp32r moving dim must be even; 344 + 346
C = 53
D_CHUNKS = 6  # ceil(690 / 128); last chunk is 50 wide
D_LAST = D - 5 * 128  # 50


def _apply_walrus_workarounds():
    """This walrus build allows at most one semaphore wait per instruction
    on several opcodes (Drain, Matmult/LDW). Patch Tile's tail drain to use
    standalone wait_ge instructions, and provide a post-pass that hoists
    excess waits onto InstNoOp instructions."""
    from concourse import tile, mybir

    def _patched_drain_and_barrier(self, tick_clock, wait_clock):
        gc = tick_clock.global_clock
        ticks = [int(s) for s in re.findall(r"\d+", repr(gc))]
        allocated = self.sems.allocated()
        for proc, sem in sorted(allocated.items()):
            t = ticks[proc] if proc < len(ticks) else 0
            if t > 0:
                mult = 16 if "DMA" in sem.name else 1
                self.nc.sync.wait_ge(sem, t * mult)
        self.nc.sync.drain()
        self.nc.all_engine_barrier()
        popped = self.nc._tile_sem_poison_stack.pop()
        assert popped is self._sem_poison
        self.nc.clear_and_free_semaphores(list(allocated.values()))
        self.nc.all_engine_barrier()

    tile.TileContext._drain_and_barrier = _patched_drain_and_barrier

    def split_multi_waits(nc, max_waits=1):
        for f in nc.m.functions:
            for b in f.blocks:
                insts = list(b.instructions)
                new = []
                dirty = False
                for inst in insts:
                    si = inst.sync_info
                    if si is not None and len(si.on_wait) > max_waits:
                        waits = list(si.on_wait)
                        extra, keep = waits[:-max_waits], waits[-max_waits:]
                        for k, w in enumerate(extra):
                            nop = mybir.InstNoOp(
                                name=f"{inst.name}-hw{k}", ins=[], outs=[]
                            )
                            nop.engine = inst.engine
                            nop.sync_info = mybir.SyncInfo(
                                on_wait=[w], on_update=[]
                            )
                            new.append(nop)
                        inst.sync_info = mybir.SyncInfo(
                            on_wait=keep, on_update=list(si.on_update)
                        )
                        dirty = True
                    new.append(inst)
                if dirty:
                    b.instructions = new

    return split_multi_waits


def _preprocess(x, scope, n_cores=N_CORES):
    """Compute per-core padded row slices and all data-dependent side
    tensors for the SPMD program."""
    n_sent = x.shape[0]
    n_bags = scope.shape[0] - 1
    scope = np.asarray(scope, dtype=np.int64)
    counts = np.diff(scope)
    assert counts.min() >= 1
    assert counts.max() < ROWS_PER_GROUP, "a bag may span at most 2 groups"
    seg_full = np.repeat(np.arange(n_bags, dtype=np.int64), counts)

    # bag-aligned row cuts near k * n_sent / n_cores
    row_cuts = [0]
    bag_cuts = [0]
    for k in range(1, n_cores):
        t = (k * n_sent) // n_cores
        b = int(np.searchsorted(scope, t, side="right")) - 1
        bag_cuts.append(b)
        row_cuts.append(int(scope[b]))
    row_cuts.append(n_sent)
    bag_cuts.append(n_bags)

    rows_per_core = [row_cuts[c + 1] - row_cuts[c] for c in range(n_cores)]
    G = int(np.ceil(max(rows_per_core) / ROWS_PER_GROUP))
    R = G * ROWS_PER_GROUP
    n_pairs = (G + 1) // 2

    cores = []
    for c in range(n_cores):
        r0, r1 = row_cuts[c], row_cuts[c + 1]
        b0, b1 = bag_cuts[c], bag_cuts[c + 1]
        nrows = r1 - r0

        x_pad = np.zeros((R, D), dtype=np.float16)
        x_pad[:nrows] = x[r0:r1].astype(np.float16)
        # [G*768, D] -> [G, 128, 6*D]: partition-major so each partition's
        # group data is one contiguous 8280B run in DRAM
        x_pad = np.ascontiguousarray(
            x_pad.reshape(G, TILES_PER_GROUP, 128, D).transpose(0, 2, 1, 3)
        ).reshape(G * 128, TILES_PER_GROUP * D)

        seg_c = seg_full[r0:r1]  # global bag ids
        # base bag per group; B1 sentinel for pad groups
        base = np.empty(G + 1, dtype=np.int64)
        for g in range(G + 1):
            rr = g * ROWS_PER_GROUP
            base[g] = seg_c[rr] if rr < nrows else b1

        seg_local = np.full(R, 128.0, dtype=np.float32)
        grp = np.arange(nrows) // ROWS_PER_GROUP
        seg_local[:nrows] = (seg_c - base[grp]).astype(np.float32)
        assert seg_local[:nrows].max(initial=0.0) <= 127.0

        fixup = np.zeros((G, 128), dtype=np.float32)
        start_col = np.zeros(G, dtype=np.int64)
        end_col = np.full(G, -1, dtype=np.int64)
        nxt_start = 0  # start col of group g computed from g-1's overlap
        for g in range(G):
            rr_end = (g + 1) * ROWS_PER_GROUP
            nb = base[g + 1]
            if g * ROWS_PER_GROUP >= nrows:
                # pad group: owns nothing
                start_col[g], end_col[g] = 1, 0
                continue
            start_col[g] = nxt_start
            if rr_end < nrows and int(scope[nb]) - r0 < rr_end:
                # bag nb has rows in both g and g+1: g owns it, fixup adds
                # g+1's partial (always at S_{g+1}[0])
                L = int(nb - base[g])
                assert 1 <= L <= 127
                fixup[g, L] = 1.0
                end_col[g] = L
                nxt_start = 1
            else:
                end_col[g] = int(nb - 1 - base[g])
                nxt_start = 0

        # recip counts per group column
        recip = np.ones((G, 128), dtype=np.float32)
        for g in range(G):
            lo = base[g]
            hi = min(lo + 128, b1)
            if hi > lo:
                recip[g, : hi - lo] = 1.0 / counts[lo:hi]

        # seg_local as [128, G*6] (col = g*6+t), recip as [128, G]
        seg_sb = seg_local.reshape(G * TILES_PER_GROUP, 128).T.copy()
        recip_sb = recip.T.copy()  # [128, G]

        cores.append(
            dict(
                x=x_pad,
                seg=np.ascontiguousarray(seg_sb),
                recip=np.ascontiguousarray(recip_sb),
                fixup=fixup.reshape(1, G * 128).copy(),
                base=base,
                start_col=start_col,
                end_col=end_col,
                b0=b0,
                b1=b1,
            )
        )
    return cores, G, n_pairs


def _build_program(G, n_pairs, rel_weight, bias):
    import concourse.bass as bass
    import concourse.mybir as mybir
    from concourse import tile

    dt = mybir.dt
    nc = bass.Bass()

    x_d = nc.declare_dram_parameter(
        "x", [G * 128, TILES_PER_GROUP * D], dt.float16, isOutput=False
    )
    seg_d = nc.declare_dram_parameter(
        "seg", [128, G * TILES_PER_GROUP], dt.float32, isOutput=False
    )
    recip_d = nc.declare_dram_parameter(
        "recip", [128, G], dt.float32, isOutput=False
    )
    fixup_d = nc.declare_dram_parameter(
        "fixup", [1, G * 128], dt.float16, isOutput=False
    )
    iota_d = nc.declare_dram_parameter("iota", [128, 128], dt.float32, isOutput=False)
    ident_d = nc.declare_dram_parameter(
        "ident", [128, 128], dt.float16, isOutput=False
    )
    wt_d = nc.declare_dram_parameter("wt", [128, 768], dt.float16, isOutput=False)
    bias_d = nc.declare_dram_parameter("bias", [C, 1], dt.float32, isOutput=False)
    out_d = nc.declare_dram_parameter(
        "out", [C, n_pairs * 256], dt.float32, isOutput=True
    )

    with tile.TileContext(nc) as tc:
        with (
            tc.tile_pool(name="const", bufs=1) as cpool,
            tc.tile_pool(name="xin", bufs=SERIAL_BUFS or 4) as xpool,
            tc.tile_pool(name="onehot", bufs=SERIAL_BUFS or 6) as apool,
            tc.tile_pool(name="rows", bufs=SERIAL_BUFS or 2) as rpool,
            tc.tile_pool(name="means", bufs=SERIAL_BUFS or 2) as mpool,
            tc.tile_pool(name="mgt", bufs=SERIAL_BUFS or 2) as tpool,
            tc.tile_pool(name="outs", bufs=2) as opool,
            tc.tile_pool(name="ps_sum", bufs=3, space="PSUM") as pspool,
            tc.tile_pool(name="ps_tr", bufs=1, space="PSUM") as ptpool,
            tc.tile_pool(name="ps_proj", bufs=1, space="PSUM") as pppool,
        ):
            iota_t = cpool.tile([128, 128], dt.float32)
            ident_t = cpool.tile([128, 128], dt.float16)
            seg_t = cpool.tile([128, G * TILES_PER_GROUP], dt.float32)
            recip_t = cpool.tile([128, G], dt.float32)
            fixup_t = cpool.tile([1, G * 128], dt.float16)
            wt_t = cpool.tile([128, 768], dt.float16)
            bias_t = cpool.tile([C, 1], dt.float32)

            nc.gpsimd.dma_start(out=iota_t[:], in_=iota_d[:])
            nc.gpsimd.dma_start(out=ident_t[:], in_=ident_d[:])
            nc.gpsimd.dma_start(out=seg_t[:], in_=seg_d[:])
            nc.gpsimd.dma_start(out=recip_t[:], in_=recip_d[:])
            nc.gpsimd.dma_start(out=fixup_t[:], in_=fixup_d[:])
            nc.gpsimd.dma_start(out=wt_t[:], in_=wt_d[:])
            nc.gpsimd.dma_start(out=bias_t[:], in_=bias_d[:])



            prev = None  # (ps_a, ps_b, g-1)
            r_tile = None
            mgt = None

            for g in range(G + 1):
                cur = None
                if g < G:
                    x_t = xpool.tile(
                        [128, TILES_PER_GROUP * D], dt.float16, tag="x"
                    )
                    nc.sync.dma_start(
                        out=x_t[:], in_=x_d[g * 128 : (g + 1) * 128, :]
                    )
                    ps_a = pspool.tile([128, D_SPLIT], dt.float32, tag="psa")
                    ps_b = pspool.tile([128, D - D_SPLIT], dt.float32, tag="psb")
                    for t in range(TILES_PER_GROUP):
                        a_t = apool.tile([128, 128], dt.float16, tag="a")
                        col = g * TILES_PER_GROUP + t
                        nc.vector.tensor_scalar(
                            out=a_t[:],
                            in0=iota_t[:],
                            scalar1=seg_t[:, col : col + 1],
                            scalar2=None,
                            op0=mybir.AluOpType.is_equal,
                        )
                        first = t == 0
                        nc.tensor.matmul(
                            ps_a[:],
                            a_t[:],
                            x_t[:, t * D : t * D + D_SPLIT],
                            start=first,
                            stop=False,
                        )
                        nc.tensor.matmul(
                            ps_b[:],
                            a_t[:],
                            x_t[:, t * D + D_SPLIT : (t + 1) * D],
                            start=first,
                            stop=False,
                        )
                    cur = (ps_a, ps_b)
                    if g >= 1:
                        # row 0 of this group's partial sums, for the fixup
                        # of group g-1 (bag split across the boundary)
                        r_tile = rpool.tile([1, D], dt.float16, tag="r")
                        nc.scalar.copy(r_tile[:, 0:D_SPLIT], ps_a[0:1, :])
                        nc.scalar.copy(r_tile[:, D_SPLIT:D], ps_b[0:1, :])

                if g >= 1:
                    pg = g - 1
                    pa, pb = prev
                    # fixup: S_{g-1}[L] += S_g[0]; zero one-hot for no-op.
                    # For g == G reuse the last r_tile (one-hot is zero).
                    fx = fixup_t[:, pg * 128 : (pg + 1) * 128]
                    nc.tensor.matmul(
                        pa[:], fx, r_tile[:, 0:D_SPLIT], start=False, stop=True
                    )
                    nc.tensor.matmul(
                        pb[:], fx, r_tile[:, D_SPLIT:D], start=False, stop=True
                    )
                    # means = psum * recip (per output column of group pg)
                    means = mpool.tile([128, D], dt.float16, tag="m")
                    nc.scalar.activation(
                        means[:, 0:D_SPLIT],
                        pa[:],
                        mybir.ActivationFunctionType.Copy,
                        scale=recip_t[:, pg : pg + 1],
                    )
                    nc.scalar.activation(
                        means[:, D_SPLIT:D],
                        pb[:],
                        mybir.ActivationFunctionType.Copy,
                        scale=recip_t[:, pg : pg + 1],
                    )
                    # wait: recip scaling must be per *partition* = bag row
                    # of the psum ([bag, D] layout) -> scalar1 is [128,1] ok

                    h = pg % 2
                    if h == 0:
                        mgt = tpool.tile([128, 6 * 256], dt.float16, tag="mgt")
                    for d in range(D_CHUNKS):
                        w = 128 if d < 5 else D_LAST
                        ps_t = ptpool.tile([128, 128], dt.float16, tag="pt")
                        nc.tensor.transpose(
                            ps_t[0:w, :],
                            means[:, d * 128 : d * 128 + w],
                            ident_t[:],
                        )
                        nc.vector.tensor_copy(
                            mgt[0:w, d * 256 + h * 128 : d * 256 + h * 128 + 128],
                            ps_t[0:w, :],
                        )
                    if h == 1 or g == G:
                        q = pg // 2
                        pp = pppool.tile([128, 256], dt.float32, tag="pp")
                        for d in range(D_CHUNKS):
                            w = 128 if d < 5 else D_LAST
                            nc.tensor.matmul(
                                pp[:],
                                wt_t[0:w, d * 128 : (d + 1) * 128],
                                mgt[0:w, d * 256 : (d + 1) * 256],
                                start=(d == 0),
                                stop=(d == D_CHUNKS - 1),
                            )
                        out_sb = opool.tile([C, 256], dt.float32, tag="o")
                        nc.scalar.activation(
                            out_sb[:],
                            pp[0:C, :],
                            mybir.ActivationFunctionType.Identity,
                            bias=bias_t[:],
                        )
                        nc.gpsimd.dma_start(
                            out=out_d[:, q * 256 : (q + 1) * 256], in_=out_sb[:]
                        )
                prev = cur
    return nc


def prepare(x, scope, rel_weight, bias):
    """Build the SPMD program + per-core input maps. Returns a dict with
    everything needed to execute and assemble the output."""
    split_multi_waits = _apply_walrus_workarounds()

    x = np.asarray(x, dtype=np.float32)
    scope_np = np.asarray(scope)
    rel_weight = np.asarray(rel_weight, dtype=np.float32)
    bias = np.asarray(bias, dtype=np.float32)
    n_bags = scope_np.shape[0] - 1

    cores, G, n_pairs = _preprocess(x, scope_np)
    nc = _build_program(G, n_pairs, rel_weight, bias)
    split_multi_waits(nc)

    iota = np.tile(np.arange(128, dtype=np.float32), (128, 1))
    ident = np.eye(128, dtype=np.float16)
    wt = np.zeros((128, 768), dtype=np.float16)
    wpad = np.zeros((C, 768), dtype=np.float32)
    wpad[:, :D] = rel_weight
    for d in range(6):
        wt[:, d * 128 : d * 128 + C] = wpad[:, d * 128 : (d + 1) * 128].T
    bias_in = bias.reshape(C, 1).copy()

    in_maps = []
    for c in range(N_CORES):
        cd = cores[c]
        in_maps.append(
            {
                "x": cd["x"],
                "seg": cd["seg"],
                "recip": cd["recip"],
                "fixup": cd["fixup"].astype(np.float16),
                "iota": iota,
                "ident": ident,
                "wt": wt,
                "bias": bias_in,
            }
        )

    def assemble(results):
        logits_t = np.empty((C, n_bags), dtype=np.float32)
        for c in range(N_CORES):
            out = results[c]["out"]  # [C, n_pairs*256]
            cd = cores[c]
            base, s_col, e_col = cd["base"], cd["start_col"], cd["end_col"]
            for g in range(G):
                s, e = int(s_col[g]), int(e_col[g])
                if e < s:
                    continue
                col0 = 256 * (g // 2) + 128 * (g % 2)
                bag0 = int(base[g])
                logits_t[:, bag0 + s : bag0 + e + 1] = out[
                    :, col0 + s : col0 + e + 1
                ]
        return np.ascontiguousarray(logits_t.T)

    return dict(nc=nc, in_maps=in_maps, assemble=assemble, G=G, n_pairs=n_pairs)


def kernel(x, scope, rel_weight, bias):
    from concourse.bass_utils import run_bass_kernel_spmd

    p = prepare(x, scope, rel_weight, bias)
    res = run_bass_kernel_spmd(p["nc"], p["in_maps"], list(range(N_CORES)))
    return p["assemble"](res.results)



# revision 3
# speedup vs baseline: 1.3086x; 1.3086x over previous
"""Segment-mean + projection kernel for Trainium2 (8 NeuronCores, SPMD).

logits[b] = (mean of x rows in bag b) @ rel_weight.T + bias

Strategy: data-parallel over bags, two precision streams per core.

Large bags (count >= SMALL_T) go through an fp8-e4m3 path: rows are packed
into 768-row groups (3 pairs of 128-row tiles), the per-tile one-hot
matrices are precomputed on the host (exact 0/1 values in fp8) and DMA'd
alongside x, and the PE accumulates one-hot.T @ x with DoubleRow fp8
matmuls (two 128-row tiles contracted per pass at 0.5 cycles/column).
Small bags go through a classic fp16 path (256-row groups) because fp8
quantization error scales as 1/sqrt(count) and breaks the accuracy gate
for tiny bags.

Groups hold only whole bags (no bag straddles a group boundary), so there
is no fixup pass and no cross-group dependency. Per group the three
512-col PSUM bank regions are scaled by 1/count and copied to SBUF in one
activation, transposed chunk-wise by the PE into a single PSUM tile, and
copied once (768 cols) to the mgt staging buffer. Every two groups the
relation matrix is applied (6 accumulating matmuls) and bias added.
Host assembles the final [n_bags, 53] output from per-slot columns.
"""
import sys
import re

sys.path.insert(0, "/opt/trn_rl_repo")

import numpy as np
import ml_dtypes

N_CORES = 8
USE_DR = True  # DoubleRow fp8 matmuls for the large-bag stream
SMALL_T = 5  # bags with count < SMALL_T take the fp16 path
ROWS8 = 768  # rows per fp8 group: 3 DoubleRow pairs of 256
ROWS16 = 256  # rows per fp16 group: 2 tiles of 128
MAX_BAGS = 128  # output slots per group (PSUM partitions)
D = 690
SPLIT = 230  # 3 PSUM splits of 230 cols (each within one 2KB bank)
C = 53
D_CHUNKS = 6  # ceil(690 / 128); last chunk is 50 wide
D_LAST = D - 5 * 128  # 50

F8 = ml_dtypes.float8_e4m3


def _apply_walrus_workarounds():
    """This walrus build allows at most one semaphore wait per instruction
    on several opcodes (Drain, Matmult/LDW). Patch Tile's tail drain to use
    standalone wait_ge instructions, and provide a post-pass that hoists
    excess waits onto InstNoOp instructions."""
    from concourse import tile, mybir

    def _patched_drain_and_barrier(self, tick_clock, wait_clock):
        gc = tick_clock.global_clock
        ticks = [int(s) for s in re.findall(r"\d+", repr(gc))]
        allocated = self.sems.allocated()
        for proc, sem in sorted(allocated.items()):
            t = ticks[proc] if proc < len(ticks) else 0
            if t > 0:
                mult = 16 if "DMA" in sem.name else 1
                self.nc.sync.wait_ge(sem, t * mult)
        self.nc.sync.drain()
        self.nc.all_engine_barrier()
        popped = self.nc._tile_sem_poison_stack.pop()
        assert popped is self._sem_poison
        self.nc.clear_and_free_semaphores(list(allocated.values()))
        self.nc.all_engine_barrier()

    tile.TileContext._drain_and_barrier = _patched_drain_and_barrier

    def split_multi_waits(nc, max_waits=1):
        for f in nc.m.functions:
            for b in f.blocks:
                insts = list(b.instructions)
                new = []
                dirty = False
                for inst in insts:
                    si = inst.sync_info
                    if si is not None and len(si.on_wait) > max_waits:
                        waits = list(si.on_wait)
                        extra, keep = waits[:-max_waits], waits[-max_waits:]
                        for k, w in enumerate(extra):
                            nop = mybir.InstNoOp(
                                name=f"{inst.name}-hw{k}", ins=[], outs=[]
                            )
                            nop.engine = inst.engine
                            nop.sync_info = mybir.SyncInfo(
                                on_wait=[w], on_update=[]
                            )
                            new.append(nop)
                        inst.sync_info = mybir.SyncInfo(
                            on_wait=keep, on_update=list(si.on_update)
                        )
                        dirty = True
                    new.append(inst)
                if dirty:
                    b.instructions = new

    return split_multi_waits


def _pack(cnts, max_rows, max_bags):
    """Greedy: consecutive bags into groups of <= max_rows rows and
    < max_bags bags. Returns per-bag group id and group count."""
    n = len(cnts)
    if n == 0:
        return np.zeros(0, np.int64), 0
    gids = np.zeros(n, np.int64)
    g = 0
    rows = 0
    nb = 0
    for i in range(n):
        c = int(cnts[i])
        if rows + c > max_rows or nb >= max_bags:
            g += 1
            rows = 0
            nb = 0
        gids[i] = g
        rows += c
        nb += 1
    return gids, g + 1


def _stream_arrays(x, scope, stream_bags, counts, n_groups, rows_per_group,
                   tiles_per_group, dt_np):
    """Build padded x + one-hot arrays for one stream of one core.

    Returns (xoh [n_groups*128, tiles*D(+tiles... )], recip [n_groups,128],
    slot2bag [n_groups,128]). Layout per group: row slot r -> tile
    t = r // 128, partition p = r % 128; partition line = concat over
    tiles of x row data, then concat over tiles of one-hot rows.
    """
    nb = len(stream_bags)
    gids, ng = _pack(counts[stream_bags], rows_per_group, MAX_BAGS)
    assert ng <= n_groups
    X = np.zeros((n_groups, rows_per_group, D), dtype=dt_np)
    OH = np.zeros((n_groups, rows_per_group, 128), dtype=dt_np)
    recip = np.ones((n_groups, 128), dtype=np.float32)
    slot2bag = np.full((n_groups, 128), -1, dtype=np.int64)
    if nb:
        first = np.searchsorted(gids, np.arange(ng))
        slot_of_bag = np.arange(nb) - first[gids]
        cnts = counts[stream_bags]
        rows_per_g = np.bincount(gids, weights=cnts, minlength=ng).astype(np.int64)
        row_start_g = np.concatenate([[0], np.cumsum(rows_per_g)])[:-1]
        # per-row indices
        g_of_bag = gids
        row_bag_rank = np.repeat(np.arange(nb), cnts)
        g_of_row = g_of_bag[row_bag_rank]
        n_rows = int(cnts.sum())
        row_rank = np.arange(n_rows) - row_start_g[g_of_row]
        # global row ids: rows of bag b are scope[b]:scope[b+1]
        bag_row0 = scope[stream_bags]
        within = np.arange(n_rows) - np.repeat(
            np.concatenate([[0], np.cumsum(cnts)])[:-1], cnts
        )
        grows = (bag_row0[row_bag_rank] + within).astype(np.int64)
        X[g_of_row, row_rank] = x[grows].astype(dt_np)
        OH[g_of_row, row_rank, slot_of_bag[row_bag_rank]] = 1.0
        recip[g_of_bag, slot_of_bag] = (1.0 / cnts).astype(np.float32)
        slot2bag[g_of_bag, slot_of_bag] = stream_bags
    tg = tiles_per_group
    Xr = np.ascontiguousarray(
        X.reshape(n_groups, tg, 128, D).transpose(0, 2, 1, 3)
    ).reshape(n_groups * 128, tg * D)
    OHr = np.ascontiguousarray(
        OH.reshape(n_groups, tg, 128, 128).transpose(0, 2, 1, 3)
    ).reshape(n_groups * 128, tg * 128)
    xoh = np.concatenate([Xr, OHr], axis=1)
    return np.ascontiguousarray(xoh), recip, slot2bag


def _preprocess(x, scope, n_cores=N_CORES):
    n_sent = x.shape[0]
    n_bags = scope.shape[0] - 1
    scope = np.asarray(scope, dtype=np.int64)
    counts = np.diff(scope)
    assert counts.min() >= 1
    assert counts.max() <= ROWS16, "a small bag must fit a 256-row group"

    # bag-aligned row cuts near k * n_sent / n_cores
    bag_cuts = [0]
    for k in range(1, n_cores):
        t = (k * n_sent) // n_cores
        b = int(np.searchsorted(scope, t, side="right")) - 1
        bag_cuts.append(b)
    bag_cuts.append(n_bags)

    small = counts < SMALL_T
    per_core = []
    for c in range(n_cores):
        b0, b1 = bag_cuts[c], bag_cuts[c + 1]
        bag_ids = np.arange(b0, b1)
        sb = small[b0:b1]
        large_bags = bag_ids[~sb]
        small_bags = bag_ids[sb]
        _, ng8 = _pack(counts[large_bags], ROWS8, MAX_BAGS)
        _, ng16 = _pack(counts[small_bags], ROWS16, MAX_BAGS)
        per_core.append((large_bags, small_bags, ng8, ng16))

    G8 = max(p[2] for p in per_core)
    G16 = max(p[3] for p in per_core)
    if (G8 + G16) % 2:
        G16 += 1
    G = G8 + G16

    cores = []
    for c in range(n_cores):
        large_bags, small_bags, _, _ = per_core[c]
        xoh8, recip8, s2b8 = _stream_arrays(
            x, scope, large_bags, counts, G8, ROWS8, 6, F8
        )
        xoh16, recip16, s2b16 = _stream_arrays(
            x, scope, small_bags, counts, G16, ROWS16, 2, np.float16
        )
        recip = np.concatenate([recip8, recip16], axis=0)  # [G, 128]
        slot2bag = np.concatenate([s2b8, s2b16], axis=0)  # [G, 128]
        cores.append(
            dict(
                xoh8=xoh8,
                xoh16=xoh16,
                recip=np.ascontiguousarray(recip.T),  # [128, G]
                slot2bag=slot2bag.reshape(-1),
            )
        )
    return cores, G8, G16


def _build_program(G8, G16, trn=None):
    import concourse.bass as bass
    import concourse.mybir as mybir
    from concourse import tile

    dt = mybir.dt
    G = G8 + G16
    nc = bass.Bass()

    W8 = 6 * D + 6 * 128  # 4908 bytes per partition line (fp8)
    W16 = 2 * D + 2 * 128  # 1636 fp16 elements per line
    xoh8_d = nc.declare_dram_parameter(
        "xoh8", [G8 * 128, W8], dt.float8e4, isOutput=False
    )
    xoh16_d = nc.declare_dram_parameter(
        "xoh16", [G16 * 128, W16], dt.float16, isOutput=False
    )
    recip_d = nc.declare_dram_parameter("recip", [128, G], dt.float32, isOutput=False)
    ident_d = nc.declare_dram_parameter("ident", [128, 128], dt.float16, isOutput=False)
    wt_d = nc.declare_dram_parameter("wt", [128, 768], dt.float16, isOutput=False)
    bias_d = nc.declare_dram_parameter("bias", [C, 1], dt.float32, isOutput=False)
    out_d = nc.declare_dram_parameter("out", [C, G * 128], dt.float32, isOutput=True)

    DR = mybir.MatmulPerfMode.DoubleRow if USE_DR else None

    with tile.TileContext(nc) as tc:
        with (
            tc.tile_pool(name="const", bufs=1) as cpool,
            tc.tile_pool(name="x8in", bufs=6) as x8pool,
            tc.tile_pool(name="x16in", bufs=3) as x16pool,
            tc.tile_pool(name="means", bufs=3) as mpool,
            tc.tile_pool(name="mgt", bufs=2) as tpool,
            tc.tile_pool(name="outs", bufs=2) as opool,
            tc.tile_pool(name="ps_sum", bufs=2, space="PSUM") as pspool,
            tc.tile_pool(name="ps_tr", bufs=1, space="PSUM") as ptpool,
            tc.tile_pool(name="ps_proj", bufs=1, space="PSUM") as pppool,
        ):
            ident_t = cpool.tile([128, 128], dt.float16)
            recip_t = cpool.tile([128, G], dt.float32)
            wt_t = cpool.tile([128, 768], dt.float16)
            bias_t = cpool.tile([C, 1], dt.float32)

            nc.gpsimd.dma_start(out=ident_t[:], in_=ident_d[:])
            nc.gpsimd.dma_start(out=recip_t[:], in_=recip_d[:])
            nc.gpsimd.dma_start(out=wt_t[:], in_=wt_d[:])
            nc.gpsimd.dma_start(out=bias_t[:], in_=bias_d[:])

            mgt = None
            for g in range(G):
                ps = pspool.tile([128, 3 * 512], dt.float32, tag="ps")
                if g < G8:
                    x_t = x8pool.tile([128, W8], dt.float8e4, tag="x8")
                    nc.sync.dma_start(
                        out=x_t[:], in_=xoh8_d[g * 128 : (g + 1) * 128, :]
                    )
                    for q in range(3):
                        oh = x_t[
                            :, 6 * D + q * 256 : 6 * D + (q + 1) * 256
                        ].rearrange("p (two m) -> p two m", two=2)
                        xr = x_t[:, q * 1380 : (q + 1) * 1380].rearrange(
                            "p (two d) -> p two d", two=2
                        )
                        for s in range(3):
                            if USE_DR:
                                nc.tensor.matmul(
                                    ps[:, s * 512 : s * 512 + SPLIT],
                                    oh,
                                    xr[:, :, s * SPLIT : (s + 1) * SPLIT],
                                    start=(q == 0),
                                    stop=(q == 2),
                                    perf_mode=DR,
                                )
                            else:
                                for j in range(2):
                                    nc.tensor.matmul(
                                        ps[:, s * 512 : s * 512 + SPLIT],
                                        oh[:, j, :],
                                        xr[:, j, s * SPLIT : (s + 1) * SPLIT],
                                        start=(q == 0 and j == 0),
                                        stop=(q == 2 and j == 1),
                                    )
                else:
                    gg = g - G8
                    x_t = x16pool.tile([128, W16], dt.float16, tag="x16")
                    nc.sync.dma_start(
                        out=x_t[:], in_=xoh16_d[gg * 128 : (gg + 1) * 128, :]
                    )
                    for j in range(2):
                        oh = x_t[:, 2 * D + j * 128 : 2 * D + (j + 1) * 128]
                        for s in range(3):
                            nc.tensor.matmul(
                                ps[:, s * 512 : s * 512 + SPLIT],
                                oh,
                                x_t[:, j * D + s * SPLIT : j * D + (s + 1) * SPLIT],
                                start=(j == 0),
                                stop=(j == 1),
                            )

                # means = psum * (1/count); one activation over the 3 banks
                means = mpool.tile([128, D], dt.float16, tag="m")
                ps3 = ps.rearrange("p (three b) -> p three b", three=3)[
                    :, :, 0:SPLIT
                ]
                nc.scalar.activation(
                    means[:],
                    ps3,
                    mybir.ActivationFunctionType.Copy,
                    scale=recip_t[:, g : g + 1],
                )

                # transpose into one [128, 768] fp16 psum tile
                pt = ptpool.tile([128, 768], dt.float16, tag="pt")
                for d in range(D_CHUNKS):
                    w = 128 if d < 5 else D_LAST
                    nc.tensor.transpose(
                        pt[0:w, d * 128 : d * 128 + 128],
                        means[:, d * 128 : d * 128 + w],
                        ident_t[:],
                    )
                h = g % 2
                if h == 0:
                    mgt = tpool.tile([128, 2 * 768], dt.float16, tag="mgt")
                nc.vector.tensor_copy(
                    mgt[:, h * 768 : h * 768 + 640], pt[:, 0:640]
                )
                nc.vector.tensor_copy(
                    mgt[0:D_LAST, h * 768 + 640 : (h + 1) * 768],
                    pt[0:D_LAST, 640:768],
                )

                if h == 1:
                    q2 = g // 2
                    pp = pppool.tile([128, 256], dt.float32, tag="pp")
                    mgt3 = mgt.rearrange("p (two c) -> p two c", two=2)
                    for d in range(D_CHUNKS):
                        w = 128 if d < 5 else D_LAST
                        nc.tensor.matmul(
                            pp[:],
                            wt_t[0:w, d * 128 : (d + 1) * 128],
                            mgt3[0:w, :, d * 128 : d * 128 + 128],
                            start=(d == 0),
                            stop=(d == D_CHUNKS - 1),
                        )
                    out_sb = opool.tile([C, 256], dt.float32, tag="o")
                    nc.scalar.activation(
                        out_sb[:],
                        pp[0:C, :],
                        mybir.ActivationFunctionType.Identity,
                        bias=bias_t[:],
                    )
                    nc.gpsimd.dma_start(
                        out=out_d[:, q2 * 256 : (q2 + 1) * 256], in_=out_sb[:]
                    )
    return nc


def prepare(x, scope, rel_weight, bias):
    """Build the SPMD program + per-core input maps. Returns a dict with
    everything needed to execute and assemble the output."""
    split_multi_waits = _apply_walrus_workarounds()

    x = np.asarray(x, dtype=np.float32)
    scope_np = np.asarray(scope)
    rel_weight = np.asarray(rel_weight, dtype=np.float32)
    bias = np.asarray(bias, dtype=np.float32)
    n_bags = scope_np.shape[0] - 1

    cores, G8, G16 = _preprocess(x, scope_np)
    nc = _build_program(G8, G16)
    split_multi_waits(nc)

    ident = np.eye(128, dtype=np.float16)
    wt = np.zeros((128, 768), dtype=np.float16)
    wpad = np.zeros((C, 768), dtype=np.float32)
    wpad[:, :D] = rel_weight
    for d in range(6):
        wt[:, d * 128 : d * 128 + C] = wpad[:, d * 128 : (d + 1) * 128].T
    bias_in = bias.reshape(C, 1).copy()

    in_maps = []
    for c in range(N_CORES):
        cd = cores[c]
        in_maps.append(
            {
                "xoh8": cd["xoh8"],
                "xoh16": cd["xoh16"],
                "recip": cd["recip"],
                "ident": ident,
                "wt": wt,
                "bias": bias_in,
            }
        )

    def assemble(results):
        logits_t = np.zeros((C, n_bags), dtype=np.float32)
        for c in range(N_CORES):
            out = results[c]["out"]  # [C, G*128]
            s2b = cores[c]["slot2bag"]
            valid = s2b >= 0
            logits_t[:, s2b[valid]] = out[:, valid]
        return np.ascontiguousarray(logits_t.T)

    return dict(nc=nc, in_maps=in_maps, assemble=assemble, G8=G8, G16=G16)


def kernel(x, scope, rel_weight, bias):
    from concourse.bass_utils import run_bass_kernel_spmd

    p = prepare(x, scope, rel_weight, bias)
    res = run_bass_kernel_spmd(p["nc"], p["in_maps"], list(range(N_CORES)))
    return p["assemble"](res.results)


# revision 5
# speedup vs baseline: 1.3184x; 1.0075x over previous
"""Segment-mean + projection kernel for Trainium2 (8 NeuronCores, SPMD).

logits[b] = (mean of x rows in bag b) @ rel_weight.T + bias

Strategy: data-parallel over bags, two precision streams per core.

Large bags (count >= SMALL_T) go through an fp8-e4m3 path: rows are packed
into 768-row groups (3 pairs of 128-row tiles), the per-tile one-hot
matrices are precomputed on the host (exact 0/1 values in fp8) and DMA'd
alongside x, and the PE accumulates one-hot.T @ x with DoubleRow fp8
matmuls (two 128-row tiles contracted per pass at 0.5 cycles/column).
Small bags go through a classic fp16 path (256-row groups) because fp8
quantization error scales as 1/sqrt(count) and breaks the accuracy gate
for tiny bags.

Groups hold only whole bags (no bag straddles a group boundary), so there
is no fixup pass and no cross-group dependency. Per group the three
512-col PSUM bank regions are scaled by 1/count and copied to SBUF in one
activation, transposed chunk-wise by the PE into a single PSUM tile, and
copied once (768 cols) to the mgt staging buffer. Every two groups the
relation matrix is applied (6 accumulating matmuls) and bias added.
Host assembles the final [n_bags, 53] output from per-slot columns.
"""
import sys
import re

sys.path.insert(0, "/opt/trn_rl_repo")

import numpy as np
import ml_dtypes

N_CORES = 8
USE_DR = True  # DoubleRow fp8 matmuls for the large-bag stream
SMALL_T = 5  # bags with count < SMALL_T take the fp16 path
ROWS8 = 768  # rows per fp8 group: 3 DoubleRow pairs of 256
ROWS16 = 256  # rows per fp16 group: 2 tiles of 128
MAX_BAGS = 128  # output slots per group (PSUM partitions)
D = 690
SPLIT = 230  # 3 PSUM splits of 230 cols (each within one 2KB bank)
C = 53
D_CHUNKS = 6  # ceil(690 / 128); last chunk is 50 wide
D_LAST = D - 5 * 128  # 50

F8 = ml_dtypes.float8_e4m3


def _apply_walrus_workarounds():
    """This walrus build allows at most one semaphore wait per instruction
    on several opcodes (Drain, Matmult/LDW). Patch Tile's tail drain to use
    standalone wait_ge instructions, and provide a post-pass that hoists
    excess waits onto InstNoOp instructions."""
    from concourse import tile, mybir

    def _patched_drain_and_barrier(self, tick_clock, wait_clock):
        gc = tick_clock.global_clock
        ticks = [int(s) for s in re.findall(r"\d+", repr(gc))]
        allocated = self.sems.allocated()
        for proc, sem in sorted(allocated.items()):
            t = ticks[proc] if proc < len(ticks) else 0
            if t > 0:
                mult = 16 if "DMA" in sem.name else 1
                self.nc.sync.wait_ge(sem, t * mult)
        self.nc.sync.drain()
        self.nc.all_engine_barrier()
        popped = self.nc._tile_sem_poison_stack.pop()
        assert popped is self._sem_poison
        self.nc.clear_and_free_semaphores(list(allocated.values()))
        self.nc.all_engine_barrier()

    tile.TileContext._drain_and_barrier = _patched_drain_and_barrier

    def split_multi_waits(nc, max_waits=1):
        for f in nc.m.functions:
            for b in f.blocks:
                insts = list(b.instructions)
                new = []
                dirty = False
                for inst in insts:
                    si = inst.sync_info
                    if si is not None and len(si.on_wait) > max_waits:
                        waits = list(si.on_wait)
                        extra, keep = waits[:-max_waits], waits[-max_waits:]
                        for k, w in enumerate(extra):
                            nop = mybir.InstNoOp(
                                name=f"{inst.name}-hw{k}", ins=[], outs=[]
                            )
                            nop.engine = inst.engine
                            nop.sync_info = mybir.SyncInfo(
                                on_wait=[w], on_update=[]
                            )
                            new.append(nop)
                        inst.sync_info = mybir.SyncInfo(
                            on_wait=keep, on_update=list(si.on_update)
                        )
                        dirty = True
                    new.append(inst)
                if dirty:
                    b.instructions = new

    return split_multi_waits


def _pack(cnts, max_rows, max_bags):
    """Greedy: consecutive bags into groups of <= max_rows rows and
    < max_bags bags. Returns per-bag group id and group count."""
    n = len(cnts)
    if n == 0:
        return np.zeros(0, np.int64), 0
    gids = np.zeros(n, np.int64)
    g = 0
    rows = 0
    nb = 0
    for i in range(n):
        c = int(cnts[i])
        if rows + c > max_rows or nb >= max_bags:
            g += 1
            rows = 0
            nb = 0
        gids[i] = g
        rows += c
        nb += 1
    return gids, g + 1


def _stream_arrays(x, scope, stream_bags, counts, n_groups, rows_per_group,
                   tiles_per_group, dt_np):
    """Build padded x + one-hot arrays for one stream of one core.

    Returns (xoh [n_groups*128, tiles*D(+tiles... )], recip [n_groups,128],
    slot2bag [n_groups,128]). Layout per group: row slot r -> tile
    t = r // 128, partition p = r % 128; partition line = concat over
    tiles of x row data, then concat over tiles of one-hot rows.
    """
    nb = len(stream_bags)
    gids, ng = _pack(counts[stream_bags], rows_per_group, MAX_BAGS)
    assert ng <= n_groups
    X = np.zeros((n_groups, rows_per_group, D), dtype=dt_np)
    OH = np.zeros((n_groups, rows_per_group, 128), dtype=dt_np)
    recip = np.ones((n_groups, 128), dtype=np.float32)
    slot2bag = np.full((n_groups, 128), -1, dtype=np.int64)
    if nb:
        first = np.searchsorted(gids, np.arange(ng))
        slot_of_bag = np.arange(nb) - first[gids]
        cnts = counts[stream_bags]
        rows_per_g = np.bincount(gids, weights=cnts, minlength=ng).astype(np.int64)
        row_start_g = np.concatenate([[0], np.cumsum(rows_per_g)])[:-1]
        # per-row indices
        g_of_bag = gids
        row_bag_rank = np.repeat(np.arange(nb), cnts)
        g_of_row = g_of_bag[row_bag_rank]
        n_rows = int(cnts.sum())
        row_rank = np.arange(n_rows) - row_start_g[g_of_row]
        # global row ids: rows of bag b are scope[b]:scope[b+1]
        bag_row0 = scope[stream_bags]
        within = np.arange(n_rows) - np.repeat(
            np.concatenate([[0], np.cumsum(cnts)])[:-1], cnts
        )
        grows = (bag_row0[row_bag_rank] + within).astype(np.int64)
        X[g_of_row, row_rank] = x[grows].astype(dt_np)
        OH[g_of_row, row_rank, slot_of_bag[row_bag_rank]] = 1.0
        recip[g_of_bag, slot_of_bag] = (1.0 / cnts).astype(np.float32)
        slot2bag[g_of_bag, slot_of_bag] = stream_bags
    tg = tiles_per_group
    Xr = np.ascontiguousarray(
        X.reshape(n_groups, tg, 128, D).transpose(0, 2, 1, 3)
    ).reshape(n_groups * 128, tg * D)
    OHr = np.ascontiguousarray(
        OH.reshape(n_groups, tg, 128, 128).transpose(0, 2, 1, 3)
    ).reshape(n_groups * 128, tg * 128)
    xoh = np.concatenate([Xr, OHr], axis=1)
    return np.ascontiguousarray(xoh), recip, slot2bag


def _preprocess(x, scope, n_cores=N_CORES):
    n_sent = x.shape[0]
    n_bags = scope.shape[0] - 1
    scope = np.asarray(scope, dtype=np.int64)
    counts = np.diff(scope)
    assert counts.min() >= 1
    assert counts.max() <= ROWS16, "a small bag must fit a 256-row group"

    # bag-aligned row cuts near k * n_sent / n_cores
    bag_cuts = [0]
    for k in range(1, n_cores):
        t = (k * n_sent) // n_cores
        b = int(np.searchsorted(scope, t, side="right")) - 1
        bag_cuts.append(b)
    bag_cuts.append(n_bags)

    small = counts < SMALL_T
    per_core = []
    for c in range(n_cores):
        b0, b1 = bag_cuts[c], bag_cuts[c + 1]
        bag_ids = np.arange(b0, b1)
        sb = small[b0:b1]
        large_bags = bag_ids[~sb]
        small_bags = bag_ids[sb]
        _, ng8 = _pack(counts[large_bags], ROWS8, MAX_BAGS)
        _, ng16 = _pack(counts[small_bags], ROWS16, MAX_BAGS)
        per_core.append((large_bags, small_bags, ng8, ng16))

    G8 = max(p[2] for p in per_core)
    G16 = max(p[3] for p in per_core)
    if (G8 + G16) % 2:
        G16 += 1
    G = G8 + G16

    cores = []
    for c in range(n_cores):
        large_bags, small_bags, _, _ = per_core[c]
        xoh8, recip8, s2b8 = _stream_arrays(
            x, scope, large_bags, counts, G8, ROWS8, 6, F8
        )
        xoh16, recip16, s2b16 = _stream_arrays(
            x, scope, small_bags, counts, G16, ROWS16, 2, np.float16
        )
        recip = np.concatenate([recip8, recip16], axis=0)  # [G, 128]
        slot2bag = np.concatenate([s2b8, s2b16], axis=0)  # [G, 128]
        cores.append(
            dict(
                xoh8=xoh8,
                xoh16=xoh16,
                recip=np.ascontiguousarray(recip.T),  # [128, G]
                slot2bag=slot2bag.reshape(-1),
            )
        )
    return cores, G8, G16


def _build_program(G8, G16, trn=None):
    import concourse.bass as bass
    import concourse.mybir as mybir
    from concourse import tile

    dt = mybir.dt
    G = G8 + G16
    nc = bass.Bass()

    W8 = 6 * D + 6 * 128  # 4908 bytes per partition line (fp8)
    W16 = 2 * D + 2 * 128  # 1636 fp16 elements per line
    xoh8_d = nc.declare_dram_parameter(
        "xoh8", [G8 * 128, W8], dt.float8e4, isOutput=False
    )
    xoh16_d = nc.declare_dram_parameter(
        "xoh16", [G16 * 128, W16], dt.float16, isOutput=False
    )
    recip_d = nc.declare_dram_parameter("recip", [128, G], dt.float32, isOutput=False)
    ident_d = nc.declare_dram_parameter("ident", [128, 128], dt.float16, isOutput=False)
    wt_d = nc.declare_dram_parameter("wt", [128, 768], dt.float16, isOutput=False)
    bias_d = nc.declare_dram_parameter("bias", [C, 1], dt.float32, isOutput=False)
    out_d = nc.declare_dram_parameter("out", [C, G * 128], dt.float32, isOutput=True)

    DR = mybir.MatmulPerfMode.DoubleRow if USE_DR else None

    with tile.TileContext(nc) as tc:
        with (
            tc.tile_pool(name="const", bufs=1) as cpool,
            tc.tile_pool(name="x8in", bufs=6) as x8pool,
            tc.tile_pool(name="x16in", bufs=3) as x16pool,
            tc.tile_pool(name="means", bufs=3) as mpool,
            tc.tile_pool(name="mgt", bufs=2) as tpool,
            tc.tile_pool(name="outs", bufs=2) as opool,
            tc.tile_pool(name="ps_sum", bufs=2, space="PSUM") as pspool,
            tc.tile_pool(name="ps_tr", bufs=1, space="PSUM") as ptpool,
            tc.tile_pool(name="ps_proj", bufs=1, space="PSUM") as pppool,
        ):
            ident_t = cpool.tile([128, 128], dt.float16)
            recip_t = cpool.tile([128, G], dt.float32)
            wt_t = cpool.tile([128, 768], dt.float16)
            bias_t = cpool.tile([C, 1], dt.float32)

            nc.gpsimd.dma_start(out=ident_t[:], in_=ident_d[:])
            nc.gpsimd.dma_start(out=recip_t[:], in_=recip_d[:])
            nc.gpsimd.dma_start(out=wt_t[:], in_=wt_d[:])
            nc.gpsimd.dma_start(out=bias_t[:], in_=bias_d[:])

            mgt = None
            means_q = {}  # group -> means tile, consumed one iteration later
            for g in range(G + 1):
                if g < G:
                    ps = pspool.tile([128, 3 * 512], dt.float32, tag="ps")
                if g >= G:
                    pass
                elif g < G8:
                    x_t = x8pool.tile([128, W8], dt.float8e4, tag="x8")
                    nc.sync.dma_start(
                        out=x_t[:], in_=xoh8_d[g * 128 : (g + 1) * 128, :]
                    )
                    for q in range(3):
                        oh = x_t[
                            :, 6 * D + q * 256 : 6 * D + (q + 1) * 256
                        ].rearrange("p (two m) -> p two m", two=2)
                        xr = x_t[:, q * 1380 : (q + 1) * 1380].rearrange(
                            "p (two d) -> p two d", two=2
                        )
                        for s in range(3):
                            if USE_DR:
                                nc.tensor.matmul(
                                    ps[:, s * 512 : s * 512 + SPLIT],
                                    oh,
                                    xr[:, :, s * SPLIT : (s + 1) * SPLIT],
                                    start=(q == 0),
                                    stop=(q == 2),
                                    perf_mode=DR,
                                )
                            else:
                                for j in range(2):
                                    nc.tensor.matmul(
                                        ps[:, s * 512 : s * 512 + SPLIT],
                                        oh[:, j, :],
                                        xr[:, j, s * SPLIT : (s + 1) * SPLIT],
                                        start=(q == 0 and j == 0),
                                        stop=(q == 2 and j == 1),
                                    )
                else:
                    gg = g - G8
                    x_t = x16pool.tile([128, W16], dt.float16, tag="x16")
                    nc.sync.dma_start(
                        out=x_t[:], in_=xoh16_d[gg * 128 : (gg + 1) * 128, :]
                    )
                    for j in range(2):
                        oh = x_t[:, 2 * D + j * 128 : 2 * D + (j + 1) * 128]
                        for s in range(3):
                            nc.tensor.matmul(
                                ps[:, s * 512 : s * 512 + SPLIT],
                                oh,
                                x_t[:, j * D + s * SPLIT : j * D + (s + 1) * SPLIT],
                                start=(j == 0),
                                stop=(j == 1),
                            )

                if g < G:
                    # means = psum * (1/count); one activation over 3 banks
                    means = mpool.tile([128, D], dt.float16, tag="m")
                    ps3 = ps.rearrange("p (three b) -> p three b", three=3)[
                        :, :, 0:SPLIT
                    ]
                    nc.scalar.activation(
                        means[:],
                        ps3,
                        mybir.ActivationFunctionType.Copy,
                        scale=recip_t[:, g : g + 1],
                    )
                    means_q[g] = means

                # transpose/copy/project for the PREVIOUS group so the PE
                # never waits on this group's means activation
                t = g - 1
                if t < 0:
                    continue
                means = means_q.pop(t)
                pt = ptpool.tile([128, 768], dt.float16, tag="pt")
                for d in range(D_CHUNKS):
                    w = 128 if d < 5 else D_LAST
                    nc.tensor.transpose(
                        pt[0:w, d * 128 : d * 128 + 128],
                        means[:, d * 128 : d * 128 + w],
                        ident_t[:],
                    )
                h = t % 2
                if h == 0:
                    mgt = tpool.tile([128, 2 * 768], dt.float16, tag="mgt")
                nc.vector.tensor_copy(
                    mgt[:, h * 768 : h * 768 + 640], pt[:, 0:640]
                )
                nc.vector.tensor_copy(
                    mgt[0:D_LAST, h * 768 + 640 : (h + 1) * 768],
                    pt[0:D_LAST, 640:768],
                )

                if h == 1:
                    q2 = t // 2
                    pp = pppool.tile([128, 256], dt.float32, tag="pp")
                    mgt3 = mgt.rearrange("p (two c) -> p two c", two=2)
                    for d in range(D_CHUNKS):
                        w = 128 if d < 5 else D_LAST
                        nc.tensor.matmul(
                            pp[:],
                            wt_t[0:w, d * 128 : (d + 1) * 128],
                            mgt3[0:w, :, d * 128 : d * 128 + 128],
                            start=(d == 0),
                            stop=(d == D_CHUNKS - 1),
                        )
                    out_sb = opool.tile([C, 256], dt.float32, tag="o")
                    nc.scalar.activation(
                        out_sb[:],
                        pp[0:C, :],
                        mybir.ActivationFunctionType.Identity,
                        bias=bias_t[:],
                    )
                    nc.gpsimd.dma_start(
                        out=out_d[:, q2 * 256 : (q2 + 1) * 256], in_=out_sb[:]
                    )
    return nc


def prepare(x, scope, rel_weight, bias):
    """Build the SPMD program + per-core input maps. Returns a dict with
    everything needed to execute and assemble the output."""
    split_multi_waits = _apply_walrus_workarounds()

    x = np.asarray(x, dtype=np.float32)
    scope_np = np.asarray(scope)
    rel_weight = np.asarray(rel_weight, dtype=np.float32)
    bias = np.asarray(bias, dtype=np.float32)
    n_bags = scope_np.shape[0] - 1

    cores, G8, G16 = _preprocess(x, scope_np)
    nc = _build_program(G8, G16)
    split_multi_waits(nc)

    ident = np.eye(128, dtype=np.float16)
    wt = np.zeros((128, 768), dtype=np.float16)
    wpad = np.zeros((C, 768), dtype=np.float32)
    wpad[:, :D] = rel_weight
    for d in range(6):
        wt[:, d * 128 : d * 128 + C] = wpad[:, d * 128 : (d + 1) * 128].T
    bias_in = bias.reshape(C, 1).copy()

    in_maps = []
    for c in range(N_CORES):
        cd = cores[c]
        in_maps.append(
            {
                "xoh8": cd["xoh8"],
                "xoh16": cd["xoh16"],
                "recip": cd["recip"],
                "ident": ident,
                "wt": wt,
                "bias": bias_in,
            }
        )

    def assemble(results):
        logits_t = np.zeros((C, n_bags), dtype=np.float32)
        for c in range(N_CORES):
            out = results[c]["out"]  # [C, G*128]
            s2b = cores[c]["slot2bag"]
            valid = s2b >= 0
            logits_t[:, s2b[valid]] = out[:, valid]
        return np.ascontiguousarray(logits_t.T)

    return dict(nc=nc, in_maps=in_maps, assemble=assemble, G8=G8, G16=G16)


def kernel(x, scope, rel_weight, bias):
    from concourse.bass_utils import run_bass_kernel_spmd

    p = prepare(x, scope, rel_weight, bias)
    res = run_bass_kernel_spmd(p["nc"], p["in_maps"], list(range(N_CORES)))
    return p["assemble"](res.results)
